# revision 1
# baseline (speedup 1.0000x reference)
"""Trainium2 Bass kernel: MinEntropyConsensusLoss.

Reference computation:
    lx = log_softmax(x, axis=1); ly = log_softmax(y, axis=1)
    ce = 0.5 * (-(lx + ly)).min(axis=1)          # [N]
    out = ce.mean()                               # scalar

Identity used here:
    -(lx + ly)[n, c] = lse_x[n] + lse_y[n] - (x + y)[n, c]
    min_c(...)       = lse_x[n] + lse_y[n] - max_c(x + y)[n]
so per row only three free-dim reductions are needed:
    sum(exp(x)), sum(exp(y)), max(x + y)

v2 design (measured-rate driven; the f32 kernel was ACT-bound, not
DMA-bound — 64 fp16-out exp instructions cost 3.0us each = 192us):
  * Inputs are cast to fp16 on the host — halves HBM traffic (DMA floor
    ~93us/core at the measured ~344 GB/s/core).  fp16 quantization
    contributes ~2e-6 rel err (validated vs f64).
  * sum(exp(.)): ACT `exp` per [128,2048] block with accum_out and an
    **f32 out tile** — fp16-out exp measured 3.0us/instr vs 1.2us with
    f32 out; 64 instructions = 77us.
  * max(x+y): per-block DVE tensor_add (fp16) + reduce_max = 2.73us per
    block pair, 87us total.  Per-block 1D APs hit the DVE 16-bit fast
    path (chunk-fused 3D APs measured slower).  The fused
    tensor_tensor_reduce op raises INTERNAL errors on this stack for
    min/max reductions, so it is not used.
  * The per-row reductions stream into an on-chip [128, 96] f32
    accumulator (cols 0-31 sum_exp_x per block, 32-63 sum_exp_y,
    64-95 max(x+y)); the final ln + sum + mean runs on the host.  This
    removes the Ln pass and the exp<->ln ACT table switching entirely.
  * Optional (off at schrau=0 — DVE is the tighter engine): a DVE int16
    Schraudolph exp — z = x*1024/ln2 + (15*1024-60) converted to int16
    approximates the fp16 bit pattern of e^x, so bitcast + reduce_sum
    gives the row sums in 2 DVE passes (1.3e-4 rel err at full use).
  * x loads issue on the SP HWDGE ring, y loads on the ACT HWDGE ring:
    two independent rings hide each DMA's ~2us completion-receipt cost.

Sharding: data-parallel on N across the 8 NeuronCores (4096 rows each).
"""

import numpy as np

N, C = 32768, 2048
NCORES = 8
NPER = N // NCORES  # 4096 rows per core
P = 128             # SBUF partitions
NBLK = NPER // P    # 32 row-blocks per core

A16 = 1024.0 / 0.6931471805599453   # 1024/ln2
B16 = 15.0 * 1024.0 - 60.0          # fp16 exp bias - sawtooth centering

# chunk schedule (DMA sizes in 128-row blocks) and the set of blocks
# whose exp runs on DVE instead of ACT.
BEST = dict(q=2, io_bufs=6, s_bufs=2, rings="alt", schrau=0, taper=True,
            pf=3)

_cache: dict = {}


def _split_waits(nc, max_waits=1):
    """This container's pinned walrus encodes at most one sync-wait per
    instruction; hoist extra waits onto preceding NoOps (same engine, so
    wait-for-all semantics are preserved)."""
    from concourse import mybir

    for f in nc.m.functions:
        for blk in f.blocks:
            i = 0
            insts = blk.instructions
            while i < len(insts):
                inst = insts[i]
                si = getattr(inst, "sync_info", None)
                if si is not None and si.on_wait and len(si.on_wait) > max_waits:
                    waits = list(si.on_wait)
                    head, tail = waits[:-max_waits], waits[-max_waits:]
                    pos = i
                    for k in range(0, len(head), max_waits):
                        nop = mybir.InstNoOp(
                            name=nc.get_next_instruction_name(),
                            ins=[], outs=[],
                            engine=inst.engine,
                            sync_info=mybir.SyncInfo(
                                on_wait=head[k : k + max_waits], on_update=[]
                            ),
                        )
                        insts.insert(pos, nop)
                        pos += 1
                        i += 1
                    inst.sync_info = mybir.SyncInfo(
                        on_wait=tail, on_update=list(si.on_update)
                    )
                i += 1


def _schrau_set(schrau):
    """Evenly spread `schrau` of the NBLK blocks for the DVE exp path."""
    if not schrau:
        return frozenset()
    return frozenset(round(i * NBLK / schrau) % NBLK for i in range(schrau))


def _build_nc(reps=1, q=4, io_bufs=3, s_bufs=2, loop_n=0, taper=False,
              rings="sp+act", schrau=10, nottr=False, ttr_mode="max16",
              gp_add=0, pf=0):
    """reps>1 repeats the whole computation back-to-back; loop_n>0 wraps
    one rep in a Tile For_i dynamic loop executing loop_n times (writing
    the same output).  Both are timing-harness-only knobs; the graded
    kernel uses reps=1, loop_n=0."""
    import concourse.bacc as bacc
    import concourse.tile as tile
    from concourse import mybir

    f32 = mybir.dt.float32
    f16 = mybir.dt.float16
    i16 = mybir.dt.int16
    AF = mybir.ActivationFunctionType
    ALU = mybir.AluOpType

    nc = bacc.Bacc("TRN2", num_devices=NCORES)
    x = nc.dram_tensor("x", [NPER, C], f16, kind="ExternalInput")
    y = nc.dram_tensor("y", [NPER, C], f16, kind="ExternalInput")
    out = nc.dram_tensor("acc", [P, 3 * NBLK], f32, kind="ExternalOutput")

    gset = (frozenset(round(i * NBLK / gp_add + 0.5) % NBLK
                      for i in range(gp_add)) if gp_add else frozenset())

    with tile.TileContext(nc) as tc:
        with (
            tc.tile_pool(name="io", bufs=io_bufs) as io,
            tc.tile_pool(name="sc", bufs=s_bufs) as scp,
            tc.tile_pool(name="accp", bufs=2) as accp,
        ):
            # chunk schedule: list of chunk sizes (in blocks) summing to
            # NBLK. taper shrinks the first/last chunks so ramp-up and
            # drain expose less work.
            if isinstance(taper, (list, tuple)):
                sched = list(taper)
            elif taper:
                rem = NBLK - 2 * min(2, q)
                t = min(2, q)
                sched = [t] + [q] * (rem // q) + ([rem % q] if rem % q else []) + [t]
            else:
                sched = [q] * (NBLK // q) + ([NBLK % q] if NBLK % q else [])
            assert sum(sched) == NBLK and max(sched) <= q, sched

            xv, yv = x.ap(), y.ap()

            def engines(i):
                if rings == "sp+act":
                    return nc.sync, nc.scalar
                if rings == "sp":
                    return nc.sync, nc.sync
                if rings == "alt":
                    return (nc.sync, nc.scalar) if i % 2 == 0 else (
                        nc.scalar, nc.sync)
                if rings == "sp+gp":
                    return nc.sync, nc.gpsimd
                raise ValueError(rings)

            nchunks = len(sched)
            # schrau counts CHUNKS whose exp-sums go to DVE (Schraudolph)
            sset = (frozenset(round(i * nchunks / schrau) % nchunks
                              for i in range(schrau)) if schrau
                    else frozenset())

            starts = []
            b0 = 0
            for qk in sched:
                starts.append(b0)
                b0 += qk

            def body(rep):
                # three single-writer accumulators (sx/sy: ACT, mxy: DVE) —
                # one shared tile would serialize ACT and DVE against each
                # other through per-tile dependency tracking.
                sxa = accp.tile([P, NBLK], f32, tag="sxa")
                sya = accp.tile([P, NBLK], f32, tag="sya")
                mxa = accp.tile([P, NBLK], f32, tag="mxa")
                pending = {}

                def issue(i):
                    qk = sched[i]
                    base = starts[i] * P
                    rows = qk * P
                    # row -> (partition, block) layout: partition p holds
                    # qk consecutive rows, so each partition's DMA chunk
                    # is one contiguous qk*C*2B run (16 KB at qk=4).
                    xs = xv[base : base + rows, :].rearrange(
                        "(p q) c -> p q c", p=P)
                    ys = yv[base : base + rows, :].rearrange(
                        "(p q) c -> p q c", p=P)
                    eng_x, eng_y = engines(i)
                    x_t = io.tile([P, qk, C], f16, tag="x",
                                  padded_shape=[P, q, C])
                    eng_x.dma_start(out=x_t, in_=xs)
                    y_t = io.tile([P, qk, C], f16, tag="y",
                                  padded_shape=[P, q, C])
                    eng_y.dma_start(out=y_t, in_=ys)
                    pending[i] = (x_t, y_t)

                for i0 in range(min(pf, nchunks)):
                    issue(i0)

                for i, qk in enumerate(sched):
                    if pf:
                        if i + pf < nchunks:
                            issue(i + pf)
                    else:
                        issue(i)
                    x_t, y_t = pending.pop(i)
                    bbase = starts[i]

                    # sum(exp(.)) — whole chunk on DVE (Schraudolph) or
                    # per-block ACT exp with accum.
                    if i in sset:
                        for src, at in ((x_t, sxa), (y_t, sya)):
                            z = scp.tile([P, qk, C], i16, tag="z",
                                         padded_shape=[P, q, C])
                            nc.vector.tensor_scalar(
                                out=z, in0=src,
                                scalar1=A16, scalar2=B16,
                                op0=ALU.mult, op1=ALU.add,
                            )
                            nc.vector.reduce_sum(
                                out=at[:, bbase : bbase + qk],
                                in_=z.bitcast(f16),
                                axis=mybir.AxisListType.X,
                            )
                    else:
                        for j in range(qk):
                            b = bbase + j
                            # f32 out: fp16-out exp measured 2.5x slower
                            # (3.0us vs 1.2us per [128,2048] instruction).
                            ex = scp.tile([P, C], f32, tag="ex")
                            nc.scalar.activation(
                                out=ex, in_=x_t[:, j, :], func=AF.Exp,
                                accum_out=sxa[:, b : b + 1],
                            )
                            ey = scp.tile([P, C], f32, tag="ey")
                            nc.scalar.activation(
                                out=ey, in_=y_t[:, j, :], func=AF.Exp,
                                accum_out=sya[:, b : b + 1],
                            )

                    # max(x+y): per-block add + reduce_max (per-block 1D
                    # APs hit the DVE 16-bit fast path; chunk-fused 3D
                    # APs measured slower).
                    for j in range(qk):
                        b = bbase + j
                        s = scp.tile([P, C], f16, tag="s")
                        aeng = nc.gpsimd if b in gset else nc.vector
                        aeng.tensor_add(s, x_t[:, j, :], y_t[:, j, :])
                        nc.vector.reduce_max(
                            out=mxa[:, b : b + 1],
                            in_=s, axis=mybir.AxisListType.X,
                        )
                ov = out.ap()
                nc.sync.dma_start(out=ov[:, 0:NBLK], in_=sxa)
                nc.sync.dma_start(out=ov[:, NBLK : 2 * NBLK], in_=sya)
                nc.sync.dma_start(out=ov[:, 2 * NBLK : 3 * NBLK], in_=mxa)

            if loop_n:
                with tc.For_i(0, loop_n, 1):
                    body(0)
            else:
                for rep in range(reps):
                    body(rep)
    nc.compile()
    _split_waits(nc)
    return nc


def _get_nc():
    if "nc" not in _cache:
        _cache["nc"] = _build_nc(**BEST)
    return _cache["nc"]


def _make_in_maps(x, y):
    """x, y: [N, C] (any float dtype) -> per-core fp16 input dicts."""
    x = np.ascontiguousarray(np.asarray(x), dtype=np.float16)
    y = np.ascontiguousarray(np.asarray(y), dtype=np.float16)
    in_maps = []
    for k in range(NCORES):
        sl = slice(k * NPER, (k + 1) * NPER)
        in_maps.append({"x": x[sl], "y": y[sl]})
    return in_maps


def _finish(results, neg_mxy=False):
    """Host epilogue: acc [128, 96] per core -> scalar loss."""
    msign = 1.0 if neg_mxy else -1.0
    total = 0.0
    for r in results:
        a = np.asarray(r["acc"], dtype=np.float64)
        sx, sy, mxy = a[:, :NBLK], a[:, NBLK : 2 * NBLK], a[:, 2 * NBLK :]
        total += np.log(sx).sum() + np.log(sy).sum() + msign * mxy.sum()
    return np.float32(0.5 * total / N)


def kernel(x, y):
    import concourse.bass_utils as bass_utils

    assert np.shape(x) == (N, C) and np.shape(y) == (N, C)

    nc = _get_nc()
    res = bass_utils.run_bass_kernel_spmd(
        nc, _make_in_maps(x, y), core_ids=list(range(NCORES))
    )
    neg = (not BEST.get("nottr")) and BEST.get("ttr_mode", "max16").startswith("min")
    return _finish(res.results, neg_mxy=neg)


if __name__ == "__main__":
    rng = np.random.default_rng(0)
    x = rng.standard_normal((N, C), dtype=np.float32)
    y = rng.standard_normal((N, C), dtype=np.float32)
    got = kernel(x=x, y=y)
    lx = x - np.log(np.exp(x.astype(np.float64)).sum(1, keepdims=True))
    ly = y - np.log(np.exp(y.astype(np.float64)).sum(1, keepdims=True))
    want = (0.5 * (-(lx + ly)).min(1)).mean()
    print("kernel:", got, "numpy:", want, "rel err:", abs(got - want) / abs(want))

